# revision 7
# baseline (speedup 1.0000x reference)
"""DGM network (nn_DGMNetT) Trainium2 Bass kernel.

Math (reference):
    X  = [t, x]                       # [N, 2]
    S1 = tanh(X @ W0 + b0)            # [N, 256]
    per layer l (3 layers):
        Z  = tanh(X @ Uz + S  @ Wz + bz)
        G  = tanh(X @ Ug + S1 @ Wg + bg)
        R  = tanh(X @ Ur + S  @ Wr + br)
        Hg = tanh(X @ Uh + (S*R) @ Wh + bh)
        S  = (1-G)*Hg + Z*S
    out = S @ Wf + bf                 # [N, 1]

Kernel strategy (pure data parallel over N across 8 cores; everything
computed in TRANSPOSED layout so X^T rows are just t / x / ones vectors):

  * Halved-state trick: layers keep St = S/2 so (1-G)/2 = sigmoid(-2*u_g)
    =: C gives  St' = C*Hg + Z*St  with no extra scaling: host pre-scales
    W' = 2W for layers >= 1.  Layer 0 consumes the UNhalved S1 (its W
    unscaled, as is Wg for the C gate in every layer) and folds the /2
    into its p2 via scalar_tensor_tensor(z, 0.5, s1).
  * Biases folded into the X-side matmul by augmenting X^T with a ones
    row (K=3 stationary [u0; u1; b]).
  * fp16 operands, fp32 PSUM accumulation; tanh/sigmoid on ScalarE
    reading PSUM directly; elementwise on VectorE in fp16 (2x mode).
  * TN=1024 rows per tile: each gate's psum is [128, 2048] (4 banks =
    half of PSUM; 2 rotating slots) and drains in a single N=2048 ACT,
    halving ScalarE's per-instruction overhead (352 cyc) vs TN=512 —
    ScalarE is the binding engine (~416us busy vs PE ~395us).
  * X-side matmuls: U blocks packed per (gate, H-half) onto 4 distinct
    PE row groups (tile_position 0/32/64/96) so waves of 4 run
    concurrently on the row-tiled PE array.
  * Emission is phase-major over groups of GRP tiles (init | per layer:
    C/Z/R + S*R then Hg + update | final), with the next group's init
    phase interleaved before the current group's final phase, so every
    engine always has independent per-tile work.
"""

import numpy as np

N_TOTAL = 131072
N_CORES = 8
R_CORE = N_TOTAL // N_CORES  # 16384 rows per core
TN = 1024                    # rows per tile
T_TILES = R_CORE // TN       # 16 tiles per core
H = 256
L = 3

_CACHE = {}

# (gate, m-half) -> (row base in ua2, col base in ua2); gate order
# 0=C(G), 1=Z, 2=R, 3=Hg.  C/Z waves then R/Hg waves, each 4-way.
XPOS = {
    (0, 0): (0, 0),
    (0, 1): (32, 0),
    (1, 0): (64, 0),
    (1, 1): (96, 0),
    (2, 0): (0, 128),
    (2, 1): (32, 128),
    (3, 0): (64, 128),
    (3, 1): (96, 128),
}


def _build_program(repeat=None):
    # repeat: perf-measurement only — wraps the whole tile loop in an
    # on-device For_i so (wall(K) - wall(1))/(K-1) isolates device time
    # from the dispatch overhead. kernel() always uses repeat=None.
    from contextlib import ExitStack

    import concourse.bacc as bacc
    import concourse.mybir as mybir
    import concourse.tile as tile

    f16 = mybir.dt.float16
    f32 = mybir.dt.float32
    AF = mybir.ActivationFunctionType

    nc = bacc.Bacc("TRN2", target_bir_lowering=False)

    x3_d = nc.dram_tensor("X3", [T_TILES, 3, TN], f16, kind="ExternalInput")
    ua_d = nc.dram_tensor("Ua", [L, 128, 256], f16, kind="ExternalInput")
    w_d = nc.dram_tensor("W2", [L, 4, 2, 128, 256], f16, kind="ExternalInput")
    w0_d = nc.dram_tensor("W0a", [128, 128], f16, kind="ExternalInput")
    wf_d = nc.dram_tensor("wf2", [128, 2], f16, kind="ExternalInput")
    out_d = nc.dram_tensor("out", [T_TILES, TN], f32, kind="ExternalOutput")

    GRP = 4  # tiles per group: phases sweep the group so engines always
    # have independent per-tile work to overlap the serial per-tile chain

    with tile.TileContext(nc) as tc, ExitStack() as ctx:
        wpool = ctx.enter_context(tc.tile_pool(name="wpool", bufs=1))
        xtp = ctx.enter_context(tc.tile_pool(name="xtp", bufs=2 * GRP + 1))
        stp = ctx.enter_context(tc.tile_pool(name="stp", bufs=2))
        gp = ctx.enter_context(tc.tile_pool(name="gp", bufs=3))
        ps = ctx.enter_context(tc.tile_pool(name="ps", bufs=2, space="PSUM"))

        # ---- resident weights (w0a first: the very first init matmuls
        # need only w0a + xt) ----
        w0_sb = wpool.tile([128, 128], f16, tag="w0", name="w0_sb")
        nc.sync.dma_start(out=w0_sb[:], in_=w0_d[:])
        wf_sb = wpool.tile([128, 2], f16, tag="wfp", name="wf_sb")
        nc.sync.dma_start(out=wf_sb[:], in_=wf_d[:])
        # ua/w loads all go on the gpsimd queue (layer 0 first) so the sync
        # queue serves only w0a + the xt tiles the init matmuls need first
        ua_sb = []
        w_sb = {}
        for l in range(L):
            u = wpool.tile([128, 256], f16, tag=f"ua{l}", name=f"ua{l}")
            nc.gpsimd.dma_start(out=u[:], in_=ua_d[l])
            ua_sb.append(u)
            for g in range(4):
                for k in range(2):
                    w = wpool.tile([128, 256], f16, tag=f"w{l}{g}{k}", name=f"w{l}{g}{k}")
                    nc.gpsimd.dma_start(out=w[:], in_=w_d[l, g, k])
                    w_sb[(l, g, k)] = w

        def xmms(psg, l, g, xt):
            # X-side matmul pair for gate g (K=3, carries the bias via the
            # ones row); per (gate, m-half) on its own PE row group so
            # adjacent emissions run concurrently on HW.
            for c in range(2):
                for m in range(2):
                    rb, cb = XPOS[(g, m)]
                    nc.tensor.matmul(
                        out=psg[:, m * TN + c * 512 : m * TN + (c + 1) * 512],
                        lhsT=ua_sb[l][rb : rb + 3, cb : cb + 128],
                        rhs=xt[rb : rb + 3, c * 512 : (c + 1) * 512],
                        start=True,
                        stop=False,
                        tile_position=(rb, 0),
                    )

        def wmms(psg, l, g, src):
            for m in range(2):
                for c in range(2):
                    for k in range(2):
                        nc.tensor.matmul(
                            out=psg[:, m * TN + c * 512 : m * TN + (c + 1) * 512],
                            lhsT=w_sb[(l, g, k)][:, m * 128 : (m + 1) * 128],
                            rhs=src[:, k * TN + c * 512 : k * TN + (c + 1) * 512],
                            start=False,
                            stop=(k == 1),
                            tile_position=(0, 0),
                        )

        def emit_init(tiles):
            xts, s1s, scur, srs = {}, {}, {}, {}
            # phase I: load X, init state
            for t in tiles:
                xt = xtp.tile([128, TN], f16, tag="xt", name=f"xt{t}")
                for g in range(4):
                    nc.sync.dma_start(out=xt[32 * g : 32 * g + 3, :], in_=x3_d[t])
                xts[t] = xt
                psi = ps.tile([128, 2 * TN], f32, tag="ps", name=f"psi{t}")
                # one wave of 4: (m, c) chunks on distinct row groups
                for m in range(2):
                    for c in range(2):
                        rb = 32 * (2 * m + c)
                        nc.tensor.matmul(
                            out=psi[:, m * TN + c * 512 : m * TN + (c + 1) * 512],
                            lhsT=w0_sb[rb : rb + 3, 0:128],
                            rhs=xt[rb : rb + 3, c * 512 : (c + 1) * 512],
                            start=True,
                            stop=True,
                            tile_position=(rb, 0),
                        )
                s1 = stp.tile([128, 2 * TN], f16, tag="s1", bufs=GRP + 1, name=f"s1_{t}")
                nc.scalar.activation(out=s1[:], in_=psi[:], func=AF.Tanh)
                s1s[t] = s1
                scur[t] = s1

            return xts, s1s, scur, srs

        def emit_layers(state):
            xts, s1s, scur, srs = state
            tiles = list(xts)
            for l in range(L):
                # phase A: C/Z/R gates + S*R
                for t in tiles:
                    xt, s1, sc = xts[t], s1s[t], scur[t]
                    psc = ps.tile([128, 2 * TN], f32, tag="ps", name=f"psc{t}_{l}")
                    psz = ps.tile([128, 2 * TN], f32, tag="ps", name=f"psz{t}_{l}")
                    # joint C/Z X-waves: 4 distinct row groups per wave
                    for cc in range(2):
                        for psg, g in ((psc, 0), (psz, 1)):
                            for m in range(2):
                                rb, cb = XPOS[(g, m)]
                                nc.tensor.matmul(
                                    out=psg[:, m * TN + cc * 512 : m * TN + (cc + 1) * 512],
                                    lhsT=ua_sb[l][rb : rb + 3, cb : cb + 128],
                                    rhs=xt[rb : rb + 3, cc * 512 : (cc + 1) * 512],
                                    start=True,
                                    stop=False,
                                    tile_position=(rb, 0),
                                )
                    wmms(psc, l, 0, s1)
                    c = gp.tile([128, 2 * TN], f16, tag="c", bufs=GRP + 1, name=f"c{t}_{l}")
                    nc.scalar.activation(out=c[:], in_=psc[:], func=AF.Sigmoid, scale=-2.0)
                    wmms(psz, l, 1, sc)
                    z = gp.tile([128, 2 * TN], f16, tag="z", name=f"z{t}_{l}")
                    nc.scalar.activation(out=z[:], in_=psz[:], func=AF.Tanh)
                    psr = ps.tile([128, 2 * TN], f32, tag="ps", name=f"psr{t}_{l}")
                    xmms(psr, l, 2, xt)
                    wmms(psr, l, 2, sc)
                    r = gp.tile([128, 2 * TN], f16, tag="r", name=f"r{t}_{l}")
                    nc.scalar.activation(out=r[:], in_=psr[:], func=AF.Tanh)
                    sr = gp.tile([128, 2 * TN], f16, tag="sr", bufs=GRP + 1, name=f"sr{t}_{l}")
                    nc.vector.tensor_mul(sr[:], sc[:], r[:])
                    srs[t] = sr
                    p2 = gp.tile([128, 2 * TN], f16, tag="p2", bufs=GRP + 1, name=f"p2_{t}_{l}")
                    if l == 0:
                        # layer-0 state is the UNhalved S1; fold the /2 here
                        nc.vector.scalar_tensor_tensor(
                            p2[:], z[:], 0.5, sc[:],
                            op0=mybir.AluOpType.mult, op1=mybir.AluOpType.mult,
                        )
                    else:
                        nc.vector.tensor_mul(p2[:], z[:], sc[:])
                    scur[t] = (c, p2)  # stash for phase B
                # phase B: Hg gate + state update
                for t in tiles:
                    xt = xts[t]
                    c, p2 = scur[t]
                    sr = srs[t]
                    psh = ps.tile([128, 2 * TN], f32, tag="ps", name=f"psh{t}_{l}")
                    xmms(psh, l, 3, xt)
                    wmms(psh, l, 3, sr)
                    h = gp.tile([128, 2 * TN], f16, tag="h", name=f"h{t}_{l}")
                    nc.scalar.activation(out=h[:], in_=psh[:], func=AF.Tanh)
                    p1 = gp.tile([128, 2 * TN], f16, tag="p1", name=f"p1_{t}_{l}")
                    nc.vector.tensor_mul(p1[:], c[:], h[:])
                    snew = stp.tile(
                        [128, 2 * TN], f16, tag="st", bufs=GRP + 1, name=f"s{t}_{l}"
                    )
                    nc.vector.tensor_add(snew[:], p1[:], p2[:])
                    scur[t] = snew

        def emit_final(state):
            xts, s1s, scur, srs = state
            tiles = list(xts)
            # phase F: final projection out = St3 @ (2*Wf)
            for t in tiles:
                fp = ps.tile([1, TN], f32, tag="ps", name=f"fp{t}")
                for c in range(2):
                    for k in range(2):
                        nc.tensor.matmul(
                            out=fp[0:1, c * 512 : (c + 1) * 512],
                            lhsT=wf_sb[:, k : k + 1],
                            rhs=scur[t][:, k * TN + c * 512 : k * TN + (c + 1) * 512],
                            start=(k == 0),
                            stop=(k == 1),
                            tile_position=(0, 0),
                        )
                ocp = gp.tile([1, TN], f32, tag="ocp", name=f"ocp{t}")
                nc.vector.tensor_copy(ocp[:], fp[:])
                # gpsimd queue: on the sync queue this would head-of-line
                # block the next group's xt loads (queues execute in order)
                nc.gpsimd.dma_start(out=out_d[t : t + 1, :], in_=ocp[0:1, :])

        def emit_tiles():
            groups = [range(g0, g0 + GRP) for g0 in range(0, T_TILES, GRP)]
            # interleave: next group's init phase is emitted before this
            # group's final phase, so PE has init matmuls to chew on while
            # the last layer's activations drain
            state = emit_init(groups[0])
            for gi in range(len(groups)):
                emit_layers(state)
                nxt = emit_init(groups[gi + 1]) if gi + 1 < len(groups) else None
                emit_final(state)
                state = nxt

        if repeat is not None:
            with tc.For_i(0, repeat, 1):
                emit_tiles()
        else:
            emit_tiles()

    nc.compile()
    return nc


def _pack_weights(inp):
    f16 = np.float16
    Ws = {}
    # gate order 0=C(G),1=Z,2=R,3=Hg
    u_keys = ["Ug", "Uz", "Ur", "Uh"]
    b_keys = ["bg", "bz", "br", "bh"]
    w_keys = ["Wg", "Wz", "Wr", "Wh"]

    ua = np.zeros((L, 128, 256), f16)
    for l in range(L):
        for g in range(4):
            for m in range(2):
                rb, cb = XPOS[(g, m)]
                cols = slice(cb, cb + 128)
                hs = slice(m * 128, (m + 1) * 128)
                ua[l, rb + 0, cols] = inp[u_keys[g]][l][0][hs].astype(f16)
                ua[l, rb + 1, cols] = inp[u_keys[g]][l][1][hs].astype(f16)
                ua[l, rb + 2, cols] = inp[b_keys[g]][l][0][hs].astype(f16)
    Ws["Ua"] = ua

    w2 = np.zeros((L, 4, 2, 128, 256), f16)
    for l in range(L):
        for g in range(4):
            # C's W consumes the UNhalved S1 always; layer 0's Z/R/H consume
            # the unhalved S1 too -> no 2x there; halved states elsewhere
            scale = 1.0 if (g == 0 or l == 0) else 2.0
            wl = (inp[w_keys[g]][l] * scale).astype(f16)  # [256, 256]
            for k in range(2):
                w2[l, g, k] = wl[128 * k : 128 * (k + 1), :]
    Ws["W2"] = w2

    # init dense: 4 copies of [W0_mhalf; b0_mhalf] on row groups
    # (m, c) -> row base 32*(2m+c), all in cols 0:128
    w0a = np.zeros((128, 128), f16)
    for m in range(2):
        hs = slice(m * 128, (m + 1) * 128)
        for cc in range(2):
            rb = 32 * (2 * m + cc)
            w0a[rb + 0] = inp["W0"][0][hs].astype(f16)
            w0a[rb + 1] = inp["W0"][1][hs].astype(f16)
            w0a[rb + 2] = inp["b0"][0][hs].astype(f16)
    Ws["W0a"] = w0a

    wf2 = np.zeros((128, 2), f16)
    for k in range(2):
        wf2[:, k] = (inp["Wf"][128 * k : 128 * (k + 1), 0] * 2.0).astype(f16)
    Ws["wf2"] = wf2
    return Ws


def make_in_maps(inp):
    Ws = _pack_weights(inp)
    t_all = inp["t"].reshape(N_CORES, R_CORE).astype(np.float16)
    x_all = inp["x"].reshape(N_CORES, R_CORE).astype(np.float16)
    in_maps = []
    for c in range(N_CORES):
        x3 = np.empty((T_TILES, 3, TN), np.float16)
        x3[:, 0, :] = t_all[c].reshape(T_TILES, TN)
        x3[:, 1, :] = x_all[c].reshape(T_TILES, TN)
        x3[:, 2, :] = 1.0
        in_maps.append(dict(X3=x3, **Ws))
    return in_maps


def kernel(**inputs):
    from concourse import bass_utils

    inp = {k: np.asarray(v) for k, v in inputs.items()}

    if "nc" not in _CACHE:
        _CACHE["nc"] = _build_program()
    nc = _CACHE["nc"]

    in_maps = make_in_maps(inp)

    res = bass_utils.run_bass_kernel_spmd(nc, in_maps, core_ids=list(range(N_CORES)))
    out = np.concatenate([r["out"].reshape(-1) for r in res.results])
    out = out.reshape(N_TOTAL, 1) + inp["bf"].astype(np.float32)
    return out.astype(np.float32)


# revision 13
# speedup vs baseline: 1.0355x; 1.0355x over previous
"""DGM network (nn_DGMNetT) Trainium2 Bass kernel.

Math (reference):
    X  = [t, x]                       # [N, 2]
    S1 = tanh(X @ W0 + b0)            # [N, 256]
    per layer l (3 layers):
        Z  = tanh(X @ Uz + S  @ Wz + bz)
        G  = tanh(X @ Ug + S1 @ Wg + bg)
        R  = tanh(X @ Ur + S  @ Wr + br)
        Hg = tanh(X @ Uh + (S*R) @ Wh + bh)
        S  = (1-G)*Hg + Z*S
    out = S @ Wf + bf                 # [N, 1]

Kernel strategy (pure data parallel over N across 8 cores; everything
computed in TRANSPOSED layout so X^T rows are just t / x / ones vectors):

  * Halved-state trick: layers keep St = S/2 so (1-G)/2 = sigmoid(-2*u_g)
    =: C gives  St' = C*Hg + Z*St  with no extra scaling: host pre-scales
    W' = 2W for layers >= 1.  Layer 0 consumes the UNhalved S1 (its W
    unscaled, as is Wg for the C gate in every layer) and folds the /2
    into its p2 via scalar_tensor_tensor(z, 0.5, s1).
  * Biases folded into the X-side matmul by augmenting X^T with a ones
    row (K=3 stationary [u0; u1; b]).
  * fp16 operands, fp32 PSUM accumulation; tanh/sigmoid on ScalarE
    reading PSUM directly; elementwise on VectorE in fp16 (2x mode).
  * Per 512-row tile: gate psum = [128, 2*512] (2 banks, both output
    halves) -> single ACT op per gate.  K=3 X-matmuls run 4-way
    row-tiled (tile_position 0/32/64/96), one row group per gate.
  * Emission is phase-major over groups of 8 tiles (init | per layer:
    C/Z/R+S*R then Hg+update | final), with the next group's init phase
    interleaved before the current group's final phase, so every engine
    always has independent per-tile work and the 8 PSUM banks rotate
    without stalling TensorE.
"""

import numpy as np

N_TOTAL = 131072
N_CORES = 8
R_CORE = N_TOTAL // N_CORES  # 16384 rows per core
TN = 512                     # rows per tile (matmul moving free dim)
T_TILES = R_CORE // TN       # 32 tiles per core
H = 256
L = 3

_CACHE = {}


def _build_program(repeat=None, probe=None):
    # repeat: perf-measurement only — wraps the whole tile loop in an
    # on-device For_i so (wall(K) - wall(1))/(K-1) isolates device time
    # from the dispatch overhead. kernel() always uses repeat=None.
    # probe: timing-only experiments (WRONG math): {"wmm_k": 1} halves PE
    # W-matmul work; {"act_frac": 0.5} halves ScalarE ACT work.
    probe = probe or {}
    WMM_K = probe.get("wmm_k", 2)
    ACT_FRAC = probe.get("act_frac", 1.0)
    WMM_FD = probe.get("wmm_fd", TN)
    NO_X = probe.get("no_x", False)
    W_REUSE = probe.get("w_reuse", False)
    from contextlib import ExitStack

    import concourse.bacc as bacc
    import concourse.mybir as mybir
    import concourse.tile as tile

    f16 = mybir.dt.float16
    f32 = mybir.dt.float32
    AF = mybir.ActivationFunctionType

    nc = bacc.Bacc("TRN2", target_bir_lowering=False)

    x3_d = nc.dram_tensor("X3", [T_TILES, 3, TN], f16, kind="ExternalInput")
    ua_d = nc.dram_tensor("Ua", [L, 128, 256], f16, kind="ExternalInput")
    w_d = nc.dram_tensor("W2", [L, 4, 2, 128, 256], f16, kind="ExternalInput")
    w0_d = nc.dram_tensor("W0a", [128, 256], f16, kind="ExternalInput")
    wf_d = nc.dram_tensor("wf2", [128, 2], f16, kind="ExternalInput")
    out_d = nc.dram_tensor("out", [T_TILES, TN], f32, kind="ExternalOutput")

    GRP = 8  # tiles per group: phases sweep the group so engines always
    # have independent per-tile work to overlap the serial per-tile chain

    with tile.TileContext(nc) as tc, ExitStack() as ctx:
        wpool = ctx.enter_context(tc.tile_pool(name="wpool", bufs=1))
        xtp = ctx.enter_context(tc.tile_pool(name="xtp", bufs=2 * GRP + 1))
        stp = ctx.enter_context(tc.tile_pool(name="stp", bufs=2))
        gp = ctx.enter_context(tc.tile_pool(name="gp", bufs=5))
        ps = ctx.enter_context(tc.tile_pool(name="ps", bufs=4, space="PSUM"))

        # ---- resident weights (w0a first: the very first init matmuls
        # need only w0a + xt) ----
        w0_sb = wpool.tile([128, 256], f16, tag="w0", name="w0_sb")
        nc.sync.dma_start(out=w0_sb[:], in_=w0_d[:])
        wf_sb = wpool.tile([128, 2], f16, tag="wfp", name="wf_sb")
        nc.sync.dma_start(out=wf_sb[:], in_=wf_d[:])
        # ua/w loads all go on the gpsimd queue (layer 0 first) so the sync
        # queue serves only w0a + the xt tiles the init matmuls need first
        ua_sb = []
        w_sb = {}
        for l in range(L):
            u = wpool.tile([128, 256], f16, tag=f"ua{l}", name=f"ua{l}")
            nc.gpsimd.dma_start(out=u[:], in_=ua_d[l])
            ua_sb.append(u)
            for g in range(4):
                for k in range(2):
                    w = wpool.tile([128, 256], f16, tag=f"w{l}{g}{k}", name=f"w{l}{g}{k}")
                    nc.gpsimd.dma_start(out=w[:], in_=w_d[l, g, k])
                    w_sb[(l, g, k)] = w

        # gate order: 0=C(G), 1=Z, 2=R, 3=Hg; row group of gate g = 32*g
        def xmms(psg, l, g, xt):
            # X-side matmul pair for gate g (K=3, carries the bias via the
            # ones row); same row group for both halves of one gate, distinct
            # row groups across gates -> concurrent on HW when adjacent.
            if NO_X:
                return
            for m in range(2):
                nc.tensor.matmul(
                    out=psg[:, m * TN : (m + 1) * TN],
                    lhsT=ua_sb[l][32 * g : 32 * g + 3, m * 128 : (m + 1) * 128],
                    rhs=xt[32 * g : 32 * g + 3, :],
                    start=True,
                    stop=False,
                    tile_position=(32 * g, 0),
                )

        def wmms(psg, l, g, src):
            for m in range(2):
                for k in range(WMM_K):
                    lw = (l, g, 0) if W_REUSE else (l, g, k)
                    lm = 0 if W_REUSE else m
                    nc.tensor.matmul(
                        out=psg[:, m * TN : m * TN + WMM_FD],
                        lhsT=w_sb[lw][:, lm * 128 : (lm + 1) * 128],
                        rhs=src[:, k * TN : k * TN + WMM_FD],
                        start=(NO_X and k == 0),
                        stop=(k == WMM_K - 1),
                        tile_position=(0, 0),
                    )

        AN = int(2 * TN * ACT_FRAC)

        def act(out, in_, func, scale=None):
            if scale is None:
                nc.scalar.activation(out=out[:, :AN], in_=in_[:, :AN], func=func)
            else:
                nc.scalar.activation(
                    out=out[:, :AN], in_=in_[:, :AN], func=func, scale=scale
                )

        def emit_init(tiles):
            xts, s1s, scur, srs = {}, {}, {}, {}
            # phase I: load X, init state
            for t in tiles:
                xt = xtp.tile([128, TN], f16, tag="xt", name=f"xt{t}")
                for g in range(4):
                    nc.sync.dma_start(out=xt[32 * g : 32 * g + 3, :], in_=x3_d[t])
                xts[t] = xt
                psi = ps.tile([128, 2 * TN], f32, tag="ps", name=f"psi{t}")
                for m in range(2):
                    nc.tensor.matmul(
                        out=psi[:, m * TN : (m + 1) * TN],
                        lhsT=w0_sb[32 * m : 32 * m + 3, m * 128 : (m + 1) * 128],
                        rhs=xt[32 * m : 32 * m + 3, :],
                        start=True,
                        stop=True,
                        tile_position=(32 * m, 0),
                    )
                s1 = stp.tile([128, 2 * TN], f16, tag="s1", bufs=GRP + 1, name=f"s1_{t}")
                act(s1, psi, AF.Tanh)
                s1s[t] = s1
                scur[t] = s1

            return xts, s1s, scur, srs

        def emit_layers(state):
            xts, s1s, scur, srs = state
            tiles = list(xts)
            for l in range(L):
                # phase A: C/Z/R gates + S*R
                for t in tiles:
                    xt, s1, sc = xts[t], s1s[t], scur[t]
                    psc = ps.tile([128, 2 * TN], f32, tag="ps", name=f"psc{t}_{l}")
                    psz = ps.tile([128, 2 * TN], f32, tag="ps", name=f"psz{t}_{l}")
                    psr = ps.tile([128, 2 * TN], f32, tag="ps", name=f"psr{t}_{l}")
                    # row-group-major trios (rg 0,1,2) -> concurrent on HW
                    for m in range(2 if not NO_X else 0):
                        for g, psg in ((0, psc), (1, psz), (2, psr)):
                            nc.tensor.matmul(
                                out=psg[:, m * TN : (m + 1) * TN],
                                lhsT=ua_sb[l][32 * g : 32 * g + 3, m * 128 : (m + 1) * 128],
                                rhs=xt[32 * g : 32 * g + 3, :],
                                start=True,
                                stop=False,
                                tile_position=(32 * g, 0),
                            )
                    wmms(psc, l, 0, s1)
                    c = gp.tile([128, 2 * TN], f16, tag="c", bufs=GRP + 1, name=f"c{t}_{l}")
                    act(c, psc, AF.Sigmoid, scale=-2.0)
                    wmms(psz, l, 1, sc)
                    z = gp.tile([128, 2 * TN], f16, tag="z", bufs=GRP + 1, name=f"z{t}_{l}")
                    act(z, psz, AF.Tanh)
                    wmms(psr, l, 2, sc)
                    r = gp.tile([128, 2 * TN], f16, tag="r", name=f"r{t}_{l}")
                    act(r, psr, AF.Tanh)
                    sr = gp.tile([128, 2 * TN], f16, tag="sr", bufs=GRP + 1, name=f"sr{t}_{l}")
                    nc.vector.tensor_mul(sr[:], sc[:], r[:])
                    srs[t] = sr
                    p2 = gp.tile([128, 2 * TN], f16, tag="p2", bufs=GRP + 1, name=f"p2_{t}_{l}")
                    if l == 0:
                        # layer-0 state is the UNhalved S1; fold the /2 here
                        nc.vector.scalar_tensor_tensor(
                            p2[:], z[:], 0.5, sc[:],
                            op0=mybir.AluOpType.mult, op1=mybir.AluOpType.mult,
                        )
                    else:
                        nc.vector.tensor_mul(p2[:], z[:], sc[:])
                    scur[t] = (c, p2)  # stash for phase B
                # phase B: Hg gate + state update
                for t in tiles:
                    xt = xts[t]
                    c, p2 = scur[t]
                    sr = srs[t]
                    psh = ps.tile([128, 2 * TN], f32, tag="ps", name=f"psh{t}_{l}")
                    xmms(psh, l, 3, xt)
                    wmms(psh, l, 3, sr)
                    h = gp.tile([128, 2 * TN], f16, tag="h", name=f"h{t}_{l}")
                    act(h, psh, AF.Tanh)
                    p1 = gp.tile([128, 2 * TN], f16, tag="p1", name=f"p1_{t}_{l}")
                    nc.vector.tensor_mul(p1[:], c[:], h[:])
                    snew = stp.tile(
                        [128, 2 * TN], f16, tag="st", bufs=2 * GRP + 1, name=f"s{t}_{l}"
                    )
                    nc.vector.tensor_add(snew[:], p1[:], p2[:])
                    scur[t] = snew

        def emit_final(state):
            xts, s1s, scur, srs = state
            tiles = list(xts)
            # phase F: final projection out = St3 @ (2*Wf)
            for t in tiles:
                fp = ps.tile([1, TN], f32, tag="ps", name=f"fp{t}")
                for k in range(2):
                    nc.tensor.matmul(
                        out=fp[0:1, :],
                        lhsT=wf_sb[:, k : k + 1],
                        rhs=scur[t][:, k * TN : (k + 1) * TN],
                        start=(k == 0),
                        stop=(k == 1),
                        tile_position=(0, 0),
                    )
                ocp = gp.tile([1, TN], f32, tag="ocp", name=f"ocp{t}")
                nc.vector.tensor_copy(ocp[:], fp[:])
                # gpsimd queue: on the sync queue this would head-of-line
                # block the next group's xt loads (queues execute in order)
                nc.gpsimd.dma_start(out=out_d[t : t + 1, :], in_=ocp[0:1, :])

        def emit_tiles():
            groups = [range(g0, g0 + GRP) for g0 in range(0, T_TILES, GRP)]
            # interleave: next group's init phase is emitted before this
            # group's final phase, so PE has init matmuls to chew on while
            # the last layer's activations drain
            state = emit_init(groups[0])
            for gi in range(len(groups)):
                emit_layers(state)
                nxt = emit_init(groups[gi + 1]) if gi + 1 < len(groups) else None
                emit_final(state)
                state = nxt

        if repeat is not None:
            with tc.For_i(0, repeat, 1):
                emit_tiles()
        else:
            emit_tiles()

    _elide_redundant_waits(nc)
    nc.compile()
    return nc


def _elide_redundant_waits(nc):
    """Drop semaphore waits already implied by an earlier wait on the same
    engine queue.

    Engine queues are strict FIFO, and sem-ge waits on monotonically
    increasing semaphores stay satisfied once passed — so if an earlier
    instruction on the same engine already waited for sem >= v, any later
    wait for sem >= v' <= v is a no-op that still costs the sequencer a
    50-100ns semaphore read (the dominant per-matmul overhead here: the
    Tile framework emits one wait per instruction per cross-engine dep,
    including e.g. a weights-DMA wait on every Ldweights).

    Tracking is per block, reset for a semaphore whenever a non-inc
    (loop-boundary reset) update to it appears in program order.
    """
    removed = 0
    for blk in nc.m.functions[0].blocks:
        seen = {}  # (engine, sem_id) -> max threshold waited so far
        for inst in blk.instructions:
            si = inst.sync_info
            if si is None:
                continue
            eng = str(inst.engine)
            ow = list(si.on_wait)
            keep = []
            changed = False
            for w in ow:
                if (
                    w.sync_type == "semaphore"
                    and w.wait_mode == "sem-ge-imm"
                    and w.wait_reg is None
                ):
                    key = (eng, w.id)
                    prev = seen.get(key)
                    if prev is not None and prev >= w.wait_value:
                        removed += 1
                        changed = True
                        continue
                    seen[key] = w.wait_value if prev is None else max(prev, w.wait_value)
                keep.append(w)
            if changed:
                si.on_wait = keep
            for u in si.on_update:
                if u.sync_type == "semaphore" and u.update_mode != "sem-inc":
                    for k in [k for k in seen if k[1] == u.id]:
                        del seen[k]
    return removed


def _pack_weights(inp):
    f16 = np.float16
    Ws = {}
    # gate order 0=C(G),1=Z,2=R,3=Hg
    u_keys = ["Ug", "Uz", "Ur", "Uh"]
    b_keys = ["bg", "bz", "br", "bh"]
    w_keys = ["Wg", "Wz", "Wr", "Wh"]

    ua = np.zeros((L, 128, 256), f16)
    for l in range(L):
        for g in range(4):
            ua[l, 32 * g + 0] = inp[u_keys[g]][l][0].astype(f16)
            ua[l, 32 * g + 1] = inp[u_keys[g]][l][1].astype(f16)
            ua[l, 32 * g + 2] = inp[b_keys[g]][l][0].astype(f16)
    Ws["Ua"] = ua

    w2 = np.zeros((L, 4, 2, 128, 256), f16)
    for l in range(L):
        for g in range(4):
            # C's W consumes the UNhalved S1 always; layer 0's Z/R/H consume
            # the unhalved S1 too -> no 2x there; halved states elsewhere
            scale = 1.0 if (g == 0 or l == 0) else 2.0
            wl = (inp[w_keys[g]][l] * scale).astype(f16)  # [256, 256]
            for k in range(2):
                w2[l, g, k] = wl[128 * k : 128 * (k + 1), :]
    Ws["W2"] = w2

    w0a = np.zeros((128, 256), f16)
    for base in (0, 32):
        w0a[base + 0] = inp["W0"][0].astype(f16)
        w0a[base + 1] = inp["W0"][1].astype(f16)
        w0a[base + 2] = inp["b0"][0].astype(f16)
    Ws["W0a"] = w0a

    wf2 = np.zeros((128, 2), f16)
    for k in range(2):
        wf2[:, k] = (inp["Wf"][128 * k : 128 * (k + 1), 0] * 2.0).astype(f16)
    Ws["wf2"] = wf2
    return Ws


def make_in_maps(inp):
    Ws = _pack_weights(inp)
    t_all = inp["t"].reshape(N_CORES, R_CORE).astype(np.float16)
    x_all = inp["x"].reshape(N_CORES, R_CORE).astype(np.float16)
    in_maps = []
    for c in range(N_CORES):
        x3 = np.empty((T_TILES, 3, TN), np.float16)
        x3[:, 0, :] = t_all[c].reshape(T_TILES, TN)
        x3[:, 1, :] = x_all[c].reshape(T_TILES, TN)
        x3[:, 2, :] = 1.0
        in_maps.append(dict(X3=x3, **Ws))
    return in_maps


def kernel(**inputs):
    from concourse import bass_utils

    inp = {k: np.asarray(v) for k, v in inputs.items()}

    if "nc" not in _CACHE:
        _CACHE["nc"] = _build_program()
    nc = _CACHE["nc"]

    in_maps = make_in_maps(inp)

    res = bass_utils.run_bass_kernel_spmd(nc, in_maps, core_ids=list(range(N_CORES)))
    out = np.concatenate([r["out"].reshape(-1) for r in res.results])
    out = out.reshape(N_TOTAL, 1) + inp["bf"].astype(np.float32)
    return out.astype(np.float32)

